# revision 3
# baseline (speedup 1.0000x reference)
"""CpxRBM translation-invariant log-psi kernel for 8 Trainium2 NeuronCores.

Computes sum(log(cosh(sym @ W.T))) where sym is the (4095, 4096) matrix of
circular shifts of v = 2*vis_states - 1 and W is (1024, 4096) complex64.

Strategy (shift-sharded):
  - 512 shifts per core (core 7's 512th shift is masked to a zero row, which
    contributes exactly 0 to both accumulators).
  - The shift matrix block is built ON DEVICE from a 4608-element window of
    the doubled v vector via an overlapping-stride DMA (symT[i,s] = vwin[i+s]),
    then mapped {0,1} -> {-1,+1} with one tensor_scalar op.
  - Complex matmul = two real bf16 matmuls (sym real): pre_r = symT.T @ Wr.T,
    pre_i = symT.T @ Wi.T, accumulated in fp32 PSUM over 32 k-chunks.
  - log(cosh(x+iy)) via: a = 2*cosh(x)*cos(y), b = 2*sinh(x)*sin(y),
      Re = 0.5*ln(a^2+b^2) - ln2
      Im = 2*atan(b / (sqrt(a^2+b^2) + a))   (exact principal atan2)
    with sqrt/reciprocal both computed as Exp/Ln so only two ACT table sets
    (natural_log_exp, trig) are ever loaded.
  - Per-core output: (128, 8) fp32 partial sums (4 o-quarters x {re, im});
    host combines, applies the 0.5 / -ln2*N / 2x factors, returns the scalar.
"""
import math
import numpy as np
import ml_dtypes
from contextlib import ExitStack

import concourse.bass as bass
import concourse.mybir as mybir
import concourse.tile as tile
from concourse import bacc
from concourse.bass_utils import run_bass_kernel_spmd

F32 = mybir.dt.float32
BF16 = mybir.dt.bfloat16
AF = mybir.ActivationFunctionType
ALU = mybir.AluOpType

PI = float(np.pi)
VIS_N = 4096
INP_N = 4096
OUP_N = 1024
N_CORES = 8
S_PER_CORE = 512          # shifts per core (core 7 masks its last one)
N_KCHUNK = 32             # 4096 / 128 contraction chunks
N_QUARTER = 4             # oup quarters of 256
OQ = OUP_N // N_QUARTER   # 256
WIN = S_PER_CORE + INP_N  # 4608 window elements per core

_nc_cache = None
last_results = None


def _build_nc():
    nc = bacc.Bacc("TRN2", target_bir_lowering=False, debug=False)

    vwin = nc.dram_tensor("vwin", [WIN], BF16, kind="ExternalInput")
    wrt = nc.dram_tensor("wrt", [INP_N, OUP_N], BF16, kind="ExternalInput")
    wit = nc.dram_tensor("wit", [INP_N, OUP_N], BF16, kind="ExternalInput")
    msk = nc.dram_tensor("msk", [128, 1], F32, kind="ExternalInput")
    acc = nc.dram_tensor("acc", [128, 2 * N_QUARTER], F32, kind="ExternalOutput")

    with tile.TileContext(nc) as tc, ExitStack() as ctx:
        singles = ctx.enter_context(tc.tile_pool(name="singles", bufs=1))
        wpool = ctx.enter_context(tc.tile_pool(name="wpool", bufs=4))
        ppool = ctx.enter_context(tc.tile_pool(name="ppool", bufs=2, space="PSUM"))
        stage = ctx.enter_context(tc.tile_pool(name="stage", bufs=2))

        half_pi = singles.tile([128, 1], F32)
        nc.vector.memset(half_pi, PI / 2.0)
        msk_sb = singles.tile([128, 1], F32)
        nc.sync.dma_start(out=msk_sb, in_=msk[:, :])
        acc_sb = singles.tile([128, 2 * N_QUARTER], F32)

        # symT[p, c, s] = vwin[c*128 + p + s], i.e. symT[i, s] = v[(s0+s+i) % N]
        symT = singles.tile([128, N_KCHUNK, S_PER_CORE], BF16)
        nc.sync.dma_start(
            out=symT, in_=bass.AP(vwin, 0, [[1, 128], [128, N_KCHUNK], [1, S_PER_CORE]])
        )
        # {0,1} -> {-1,+1} (exact in bf16)
        nc.vector.tensor_scalar(symT, symT, 2.0, 1.0, ALU.mult, ALU.subtract)
        # mask the 512th shift (a zero sym row contributes 0 to both sums)
        nc.vector.tensor_scalar(
            symT[:, :, S_PER_CORE - 1 : S_PER_CORE],
            symT[:, :, S_PER_CORE - 1 : S_PER_CORE],
            msk_sb, None, ALU.mult,
        )

        for q in range(N_QUARTER):
            ps = ppool.tile([128, 4, 2, OQ], F32, tag="ps")
            for c in range(N_KCHUNK):
                wr_t = wpool.tile([128, OQ], BF16, tag="wr")
                nc.sync.dma_start(
                    out=wr_t, in_=wrt[c * 128 : (c + 1) * 128, q * OQ : (q + 1) * OQ]
                )
                wi_t = wpool.tile([128, OQ], BF16, tag="wi")
                nc.sync.dma_start(
                    out=wi_t, in_=wit[c * 128 : (c + 1) * 128, q * OQ : (q + 1) * OQ]
                )
                for st in range(4):
                    lhsT = symT[:, c, st * 128 : (st + 1) * 128]
                    nc.tensor.matmul(
                        ps[:, st, 0, :], lhsT, wr_t,
                        start=(c == 0), stop=(c == N_KCHUNK - 1),
                    )
                    # start=False: bank already cleared by the r-group's
                    # start; has_written=0 makes the first write overwrite.
                    nc.tensor.matmul(
                        ps[:, st, 1, :], lhsT, wi_t,
                        start=False, stop=(c == N_KCHUNK - 1),
                        skip_group_check=True,
                    )

            # ---- elementwise log(cosh) + accumulate ----
            # 8 rotating fp32 buffers of shape (128, 4*OQ)
            xr = stage.tile([128, 4, OQ], F32, tag="xr")
            xi = stage.tile([128, 4, OQ], F32, tag="xi")
            g = stage.tile([128, 4, OQ], F32, tag="g")
            l = stage.tile([128, 4, OQ], F32, tag="l")
            sy = stage.tile([128, 4, OQ], F32, tag="sy")
            cy = stage.tile([128, 4, OQ], F32, tag="cy")
            ep = stage.tile([128, 4, OQ], F32, tag="ep")
            em = stage.tile([128, 4, OQ], F32, tag="em")

            nc.vector.tensor_copy(xr, ps[:, :, 0, :])
            nc.vector.tensor_copy(xi, ps[:, :, 1, :])

            # range-reduce y into [-pi, pi]:   u = y - 2pi*(y>pi) + 2pi*(y<-pi)
            nc.vector.tensor_scalar(g, xi, PI, 2.0 * PI, ALU.is_gt, ALU.mult)
            nc.vector.tensor_scalar(l, xi, -PI, 2.0 * PI, ALU.is_lt, ALU.mult)
            nc.vector.scalar_tensor_tensor(xi, g, -1.0, xi, ALU.mult, ALU.add)
            nc.vector.tensor_tensor(xi, xi, l, ALU.add)          # xi = u
            # cos argument fold: ca = u - 2pi*(u > pi/2)  (then +pi/2 bias in Sin)
            nc.vector.tensor_scalar(g, xi, PI / 2.0, 2.0 * PI, ALU.is_gt, ALU.mult)
            nc.vector.scalar_tensor_tensor(l, g, -1.0, xi, ALU.mult, ALU.add)  # l = ca

            nc.scalar.activation(sy, xi, AF.Sin)                  # sin(y)
            nc.scalar.activation(cy, l, AF.Sin, bias=half_pi)     # cos(y)
            nc.scalar.activation(ep, xr, AF.Exp)                  # e^x
            nc.scalar.activation(em, xr, AF.Exp, scale=-1.0)      # e^-x

            nc.vector.tensor_tensor(xr, ep, em, ALU.add)          # xr = 2cosh x
            nc.vector.tensor_tensor(xi, ep, em, ALU.subtract)     # xi = 2sinh x
            nc.vector.tensor_tensor(g, xr, cy, ALU.mult)          # g = a
            nc.vector.tensor_tensor(l, xi, sy, ALU.mult)          # l = b
            nc.vector.tensor_tensor(xr, g, g, ALU.mult)           # xr = a^2
            nc.vector.tensor_tensor(xi, l, l, ALU.mult)           # xi = b^2
            nc.vector.tensor_tensor(sy, xr, xi, ALU.add)          # sy = q = a^2+b^2

            # Re: accumulate ln(q); also r = sqrt(q) = exp(0.5 ln q)
            nc.scalar.activation(cy, sy, AF.Ln, accum_out=acc_sb[:, 2 * q : 2 * q + 1])
            nc.scalar.activation(ep, cy, AF.Exp, scale=0.5)       # ep = r
            nc.vector.tensor_tensor(em, ep, g, ALU.add)           # em = den = r + a
            # near the branch cut fp32 rounding can make den <= 0; clamp so
            # Ln stays finite (t then blows up to +-inf -> atan -> +-pi/2).
            nc.vector.tensor_scalar(em, em, 1e-20, None, ALU.max)
            nc.scalar.activation(xr, em, AF.Ln)                   # xr = ln(den)
            nc.scalar.activation(xi, xr, AF.Exp, scale=-1.0)      # xi = 1/den
            nc.vector.tensor_tensor(sy, l, xi, ALU.mult)          # sy = t = b/den
            # Im: accumulate atan(t); final Im = 2 * sum
            nc.scalar.activation(
                cy, sy, AF.Arctan, accum_out=acc_sb[:, 2 * q + 1 : 2 * q + 2]
            )

        nc.sync.dma_start(out=acc[:, :], in_=acc_sb)

    nc.finalize()
    return nc


def _get_nc():
    global _nc_cache
    if _nc_cache is None:
        _nc_cache = _build_nc()
    return _nc_cache


def kernel(vis_states: np.ndarray, weights: np.ndarray) -> np.ndarray:
    global last_results
    vis = np.asarray(vis_states).astype(np.float32)
    vv = np.concatenate([vis, vis]).astype(ml_dtypes.bfloat16)  # {0,1}, exact
    wrt = np.ascontiguousarray(np.asarray(weights).real.astype(np.float32).T).astype(
        ml_dtypes.bfloat16
    )
    wit = np.ascontiguousarray(np.asarray(weights).imag.astype(np.float32).T).astype(
        ml_dtypes.bfloat16
    )

    in_maps = []
    for c in range(N_CORES):
        s0 = c * S_PER_CORE
        m = np.ones((128, 1), np.float32)
        if c == N_CORES - 1:
            m[:] = 0.0  # mask kills free-dim column s=511 for every partition
        in_maps.append(
            {
                "vwin": np.ascontiguousarray(vv[s0 : s0 + WIN]),
                "wrt": wrt,
                "wit": wit,
                "msk": m,
            }
        )

    nc = _get_nc()
    res = run_bass_kernel_spmd(nc, in_maps, core_ids=list(range(N_CORES)))
    last_results = res

    tot_ln = 0.0
    tot_at = 0.0
    for r in res.results:
        a = r["acc"].astype(np.float64)
        tot_ln += a[:, 0::2].sum()
        tot_at += a[:, 1::2].sum()

    n_counted = N_CORES * S_PER_CORE * OUP_N  # includes the masked zero row
    real = 0.5 * tot_ln - math.log(2.0) * n_counted
    imag = 2.0 * tot_at
    return np.array(real + 1j * imag, dtype=np.complex64)
